# revision 14
# baseline (speedup 1.0000x reference)
"""Block-local sparse attention with relative position bias on 8 TRN2 NeuronCores.

Sharding: data-parallel over batch (bs=8 == n_cores). Core i computes batch i
end-to-end; weights are replicated. Inside each core the 20 attention blocks
(context_size=200, padded seq 4000) stream through SBUF in 5 segments of 800
tokens.

Hardcoded problem shapes (self-contained; no reference.py / spec.json reads):
  x (8, 3900, 512) f32, HEADS=8, DH=64, c=200, OFFSET=512.
"""

import math
import sys

import numpy as np

sys.path.insert(0, "/opt/trn_rl_repo")

HEADS = 8
DH = 64
DIM = 512
C = 200
N = 3900
NP = 4000
NB = 20
SEGS = 5
SEG_T = 800  # tokens per segment (4 blocks)
TC = 100  # token chunk (half block)
GW = 304  # padded G window width (299 -> 304)
SCALE = DH ** -0.5  # 0.125


def build_nc():
    import concourse.bass as bass
    import concourse.mybir as mybir
    import concourse.tile as tile
    from concourse import bacc

    f32 = mybir.dt.float32
    f32r = mybir.dt.float32r
    bf16 = mybir.dt.bfloat16
    Exp = mybir.ActivationFunctionType.Exp
    mult = mybir.AluOpType.mult
    add = mybir.AluOpType.add

    nc = bacc.Bacc("TRN2", target_bir_lowering=False, debug=False)

    x = nc.declare_dram_parameter("x", [NP, DIM], bf16, isOutput=False)
    wqkt = nc.declare_dram_parameter("wqkt", [DIM, 1024], bf16, isOutput=False)
    wvt = nc.declare_dram_parameter("wvt", [DIM, DIM], bf16, isOutput=False)
    woutt = nc.declare_dram_parameter("woutt", [DIM, DIM], bf16, isOutput=False)
    grevt = nc.declare_dram_parameter("grevt", [128, 2, GW], bf16, isOutput=False)
    idb = nc.declare_dram_parameter("idb", [128, 128], bf16, isOutput=False)
    boutb = nc.declare_dram_parameter("boutb", [128, DIM], f32, isOutput=False)
    y = nc.declare_dram_parameter("y", [N, DIM], f32, isOutput=True)

    # DRAM scratch for the relative-position shear (one slot per block/head/ihalf)
    gsc = nc.dram_tensor("gscratch", [NB, 16 * TC * GW], bf16)

    with tile.TileContext(nc) as tc:
        with (
            tc.tile_pool(name="const", bufs=1) as cpool,
            tc.tile_pool(name="seg1", bufs=1) as spool1,
            tc.tile_pool(name="seg2", bufs=2) as spool2,
            tc.tile_pool(name="work", bufs=4) as wpool,
            tc.tile_pool(name="blk", bufs=2) as bpool,
            tc.tile_pool(name="psum", bufs=8, space="PSUM") as pspool,
        ):
            # ---- constants ----
            wqk_sb = cpool.tile([128, 4, 1024], bf16, tag="wqk")
            wvt_sb = cpool.tile([128, 4, DIM], bf16, tag="wvt")
            wout_sb = cpool.tile([128, 4, DIM], bf16, tag="wout")
            grev_sb = cpool.tile([128, 2, GW], bf16, tag="grev")
            idb_sb = cpool.tile([128, 128], bf16, tag="idb")
            bout_sb = cpool.tile([128, DIM], f32, tag="bout")

            for dc in range(4):
                r = slice(dc * 128, (dc + 1) * 128)
                nc.sync.dma_start(out=wqk_sb[:, dc, :], in_=wqkt[r, :])
                nc.sync.dma_start(out=wvt_sb[:, dc, :], in_=wvt[r, :])
                nc.sync.dma_start(out=wout_sb[:, dc, :], in_=woutt[r, :])
            nc.sync.dma_start(out=grev_sb[:], in_=grevt[:])
            nc.sync.dma_start(out=idb_sb[:], in_=idb[:])
            nc.sync.dma_start(out=bout_sb[:], in_=boutb[:])

            for s in range(SEGS):
                t0 = s * SEG_T

                # ---- x.T via hardware DMA-transpose from DRAM ----
                xt = spool1.tile([128, 4, SEG_T], bf16, tag="xt")
                for dc in range(4):
                    nc.sync.dma_start(
                        out=xt[:, dc, :],
                        in_=x[t0 : t0 + SEG_T, dc * 128 : (dc + 1) * 128],
                        transpose=True,
                    )

                # ---- Q.T / K.T projections (weights stationary, fp32r) ----
                qt = spool1.tile([128, 4, SEG_T], bf16, tag="qt")
                kt = spool1.tile([128, 4, SEG_T], bf16, tag="kt")
                for hc in range(8):
                    for half in range(2):
                        ps = pspool.tile([128, 400], f32, tag="ps")
                        for dc in range(4):
                            nc.tensor.matmul(
                                ps[:],
                                lhsT=wqk_sb[:, dc, hc * 128 : (hc + 1) * 128],
                                rhs=xt[:, dc, half * 400 : (half + 1) * 400],
                                start=(dc == 0),
                                stop=(dc == 3),
                            )
                        dst = qt if hc < 4 else kt
                        nc.vector.tensor_copy(
                            dst[:, hc % 4, half * 400 : (half + 1) * 400], ps[:]
                        )

                # ---- V natural (x.T stationary, fp32r) ----
                vsb = spool1.tile([TC, 8, DIM], bf16, tag="v")
                for k in range(8):
                    ps = pspool.tile([TC, DIM], f32, tag="ps")
                    for dc in range(4):
                        nc.tensor.matmul(
                            ps[:],
                            lhsT=xt[:, dc, k * TC : (k + 1) * TC],
                            rhs=wvt_sb[:, dc, :],
                            start=(dc == 0),
                            stop=(dc == 3),
                        )
                    nc.vector.tensor_copy(vsb[:, k, :], ps[:])

                # ---- attention per (block-in-seg, head) ----
                ot = spool1.tile([128, 4, SEG_T], bf16, tag="ot")
                for up in range(4):
                    ug = s * 4 + up
                    last = ug == NB - 1
                    jw = TC if last else C  # valid key count
                    njh = 1 if last else 2

                    # G = q @ Grev.T for all (head, ihalf) of this block
                    g_sb = bpool.tile([TC, 16, GW], bf16, tag="g")
                    for h in range(HEADS):
                        hp, hr = h // 2, (h % 2) * 64
                        for ih in range(2):
                            icol = up * C + ih * TC
                            psg = pspool.tile([TC, GW], f32, tag="ps")
                            nc.tensor.matmul(
                                psg[:],
                                lhsT=qt[hr : hr + 64, hp, icol : icol + TC],
                                rhs=grev_sb[hr : hr + 64, ih, :],
                                start=True,
                                stop=True,
                            )
                            nc.vector.tensor_copy(
                                g_sb[:, h * 2 + ih, :], psg[:]
                            )
                    gd = gsc[ug]
                    nc.sync.dma_start(
                        out=gd.rearrange("(c i s) -> i c s", c=16, s=GW),
                        in_=g_sb[:],
                    )
                    # sheared read-back: pos[i', c, j] = G[i', c, 99 - i' + j]
                    pos_sb = bpool.tile([TC, 16, C], bf16, tag="pos")
                    import concourse.bass as bass_mod

                    shear = bass_mod.AP(
                        gd.tensor,
                        gd.offset + 99,
                        [[GW - 1, TC], [TC * GW, 16], [1, C]],
                    )
                    nc.sync.dma_start(out=pos_sb[:], in_=shear)

                    for h in range(HEADS):
                        hp, hr = h // 2, (h % 2) * 64
                        p_tiles = []
                        for ih in range(2):
                            icol = up * C + ih * TC
                            qslice = qt[hr : hr + 64, hp, icol : icol + TC]
                            # dots (fp32r, padded N=256)
                            psd = pspool.tile([TC, C], f32, tag="ps")
                            nc.tensor.matmul(
                                psd[:],
                                lhsT=qslice,
                                rhs=kt[hr : hr + 64, hp, up * C : (up + 1) * C],
                                start=True,
                                stop=True,
                            )
                            # logits = dots*scale + pos (pos pre-scaled on host)
                            lsb = wpool.tile([TC, C], f32, tag="L")
                            nc.vector.scalar_tensor_tensor(
                                out=lsb[:, 0:jw],
                                in0=psd[:, 0:jw],
                                scalar=SCALE,
                                in1=pos_sb[:, h * 2 + ih, 0:jw],
                                op0=mult,
                                op1=add,
                            )
                            # P = exp(logits), den = rowsum
                            psb = wpool.tile([TC, C], bf16, tag="P")
                            den = wpool.tile([TC, 1], f32, tag="den")
                            nc.scalar.activation(
                                out=psb[:, 0:jw],
                                in_=lsb[:, 0:jw],
                                func=Exp,
                                accum_out=den[:],
                            )
                            rec = wpool.tile([TC, 1], f32, tag="rec")
                            nc.vector.reciprocal(rec[:], den[:])
                            pn = wpool.tile([TC, C], bf16, tag="pn")
                            nc.gpsimd.tensor_scalar_mul(
                                pn[:, 0:jw], psb[:, 0:jw], rec[:]
                            )
                            p_tiles.append(pn)

                        # P.T with 1/den folded into the transpose identity
                        pt_tiles = []
                        for jh in range(njh):
                            pspt = pspool.tile([TC, C], bf16, tag="ps")
                            for ih in range(2):
                                nc.tensor.transpose(
                                    pspt[:, ih * TC : (ih + 1) * TC],
                                    in_=p_tiles[ih][:, jh * TC : (jh + 1) * TC],
                                    identity=idb_sb[0:TC, 0:TC],
                                )
                            ptb = wpool.tile([TC, C], bf16, tag="pt")
                            nc.vector.tensor_copy(ptb[:], pspt[:])
                            pt_tiles.append(ptb)

                        # O.T = V.T @ P.T  (accumulate over key halves)
                        pso = pspool.tile([64, C], f32, tag="ps")
                        for jh in range(njh):
                            nc.tensor.matmul(
                                pso[:],
                                lhsT=vsb[:, up * 2 + jh, h * 64 : (h + 1) * 64],
                                rhs=pt_tiles[jh][:],
                                start=(jh == 0),
                                stop=(jh == njh - 1),
                            )
                        nc.vector.tensor_copy(
                            ot[hr : hr + 64, hp, up * C : (up + 1) * C], pso[:]
                        )

                # ---- output projection (O.T stationary, fp32r) ----
                yt = spool2.tile([TC, 8, DIM], f32, tag="yt")
                nk = 8 if s < SEGS - 1 else 7
                for k in range(nk):
                    psy = pspool.tile([TC, DIM], f32, tag="ps")
                    for hdc in range(4):
                        nc.tensor.matmul(
                            psy[:],
                            lhsT=ot[:, hdc, k * TC : (k + 1) * TC],
                            rhs=wout_sb[:, hdc, :],
                            start=(hdc == 0),
                            stop=(hdc == 3),
                        )
                    nc.vector.tensor_add(yt[:, k, :], psy[:], bout_sb[0:TC, :])
                nc.sync.dma_start(
                    out=y[t0 : t0 + nk * TC, :].rearrange(
                        "(k p) d -> p k d", p=TC
                    ),
                    in_=yt[:, 0:nk, :],
                )

    nc.compile()
    return nc


def prep_inputs(x, Wq, Wkv, Wout, bout, rel_emb):
    """Host-side weight re-layouts + padding. Returns per-core in_maps."""
    import ml_dtypes

    x = np.asarray(x, dtype=np.float32)
    Wq = np.asarray(Wq, dtype=np.float32)
    Wkv = np.asarray(Wkv, dtype=np.float32)
    Wout = np.asarray(Wout, dtype=np.float32)
    bout = np.asarray(bout, dtype=np.float32)
    rel_emb = np.asarray(rel_emb, dtype=np.float32)

    bs = x.shape[0]
    bf = ml_dtypes.bfloat16
    xpad = np.zeros((bs, NP, DIM), dtype=bf)
    xpad[:, :N, :] = x.astype(bf)

    Wk = Wkv[:DIM]
    Wv = Wkv[DIM:]
    wqkt = np.ascontiguousarray(
        np.concatenate([Wq.T, Wk.T], axis=1)
    ).astype(bf)  # (512, 1024)
    wvt = np.ascontiguousarray(Wv.T).astype(bf)  # (512, 512)
    woutt = np.ascontiguousarray(Wout.T).astype(bf)  # (512, 512)

    # Grev[s] = rel_emb[711 - s] * scale; two per-ihalf windows of width GW
    grev = rel_emb[711 : 711 - 399 : -1] * SCALE  # (399, 64)
    grevt = np.zeros((128, 2, GW), dtype=bf)
    grevt[:DH, 0, :299] = grev[100:399].T
    grevt[:DH, 1, :304] = grev[0:304].T
    grevt[DH:, :, :] = grevt[:DH, :, :]

    idb = np.eye(128, dtype=np.float32).astype(ml_dtypes.bfloat16)
    boutb = np.ascontiguousarray(np.broadcast_to(bout, (128, DIM))).astype(
        np.float32
    )

    in_maps = []
    for b in range(bs):
        in_maps.append(
            dict(
                x=np.ascontiguousarray(xpad[b]),
                wqkt=wqkt,
                wvt=wvt,
                woutt=woutt,
                grevt=grevt,
                idb=idb,
                boutb=boutb,
            )
        )
    return in_maps


def kernel(x, Wq, Wkv, Wout, bout, rel_emb, context_size=200, **_):
    from concourse.bass_utils import run_bass_kernel_spmd

    in_maps = prep_inputs(x, Wq, Wkv, Wout, bout, rel_emb)
    nc = build_nc()
    res = run_bass_kernel_spmd(nc, in_maps, core_ids=list(range(8)))
    out = np.stack([res.results[b]["y"] for b in range(8)], axis=0)
    return out.astype(np.float32)


if __name__ == "__main__":
    nc = build_nc()
    print("built ok")


# revision 16
# speedup vs baseline: 1.9003x; 1.9003x over previous
"""Block-local sparse attention with relative position bias on 8 TRN2 NeuronCores.

Sharding: data-parallel over batch (bs=8 == n_cores). Core i computes batch i
end-to-end; weights are replicated. Inside each core the 20 attention blocks
(context_size=200, padded seq 4000) stream through SBUF in 5 segments of 800
tokens.

Hardcoded problem shapes (self-contained; no reference.py / spec.json reads):
  x (8, 3900, 512) f32, HEADS=8, DH=64, c=200, OFFSET=512.
"""

import math
import sys

import numpy as np

sys.path.insert(0, "/opt/trn_rl_repo")

HEADS = 8
DH = 64
DIM = 512
C = 200
N = 3900
NP = 4000
NB = 20
SEGS = 5
SEG_T = 800  # tokens per segment (4 blocks)
TC = 100  # token chunk (half block)
GW = 304  # padded G window width (299 -> 304)
SCALE = DH ** -0.5  # 0.125


def build_nc():
    import concourse.bass as bass
    import concourse.mybir as mybir
    import concourse.tile as tile
    from concourse import bacc

    f32 = mybir.dt.float32
    f32r = mybir.dt.float32r
    bf16 = mybir.dt.bfloat16
    Exp = mybir.ActivationFunctionType.Exp
    mult = mybir.AluOpType.mult
    add = mybir.AluOpType.add

    nc = bacc.Bacc("TRN2", target_bir_lowering=False, debug=False)

    x = nc.declare_dram_parameter("x", [NP, DIM], bf16, isOutput=False)
    wqkt = nc.declare_dram_parameter("wqkt", [DIM, 1024], bf16, isOutput=False)
    wvt = nc.declare_dram_parameter("wvt", [DIM, DIM], bf16, isOutput=False)
    woutt = nc.declare_dram_parameter("woutt", [DIM, DIM], bf16, isOutput=False)
    grevt = nc.declare_dram_parameter("grevt", [128, 2, GW], bf16, isOutput=False)
    idb = nc.declare_dram_parameter("idb", [128, 128], bf16, isOutput=False)
    boutb = nc.declare_dram_parameter("boutb", [128, DIM], f32, isOutput=False)
    y = nc.declare_dram_parameter("y", [N, DIM], f32, isOutput=True)

    # DRAM scratch for the relative-position shear (one slot per block/head/ihalf)
    gsc = nc.dram_tensor("gscratch", [NB, 16 * TC * GW], bf16)

    with tile.TileContext(nc) as tc:
        with (
            tc.tile_pool(name="const", bufs=1) as cpool,
            tc.tile_pool(name="seg1", bufs=2) as spool1,
            tc.tile_pool(name="seg2", bufs=2) as spool2,
            tc.tile_pool(name="work", bufs=4) as wpool,
            tc.tile_pool(name="blk", bufs=2) as bpool,
            tc.tile_pool(name="psum", bufs=8, space="PSUM") as pspool,
        ):
            # ---- constants ----
            wqk_sb = cpool.tile([128, 4, 1024], bf16, tag="wqk")
            wvt_sb = cpool.tile([128, 4, DIM], bf16, tag="wvt")
            wout_sb = cpool.tile([128, 4, DIM], bf16, tag="wout")
            grev_sb = cpool.tile([128, 2, GW], bf16, tag="grev")
            idb_sb = cpool.tile([128, 128], bf16, tag="idb")
            bout_sb = cpool.tile([128, DIM], f32, tag="bout")

            for dc in range(4):
                r = slice(dc * 128, (dc + 1) * 128)
                nc.sync.dma_start(out=wqk_sb[:, dc, :], in_=wqkt[r, :])
                nc.sync.dma_start(out=wvt_sb[:, dc, :], in_=wvt[r, :])
                nc.sync.dma_start(out=wout_sb[:, dc, :], in_=woutt[r, :])
            nc.sync.dma_start(out=grev_sb[:], in_=grevt[:])
            nc.sync.dma_start(out=idb_sb[:], in_=idb[:])
            nc.sync.dma_start(out=bout_sb[:], in_=boutb[:])

            for s in range(SEGS):
                t0 = s * SEG_T

                # ---- x.T via hardware DMA-transpose from DRAM ----
                xt = spool1.tile([128, 4, SEG_T], bf16, tag="xt")
                for dc in range(4):
                    nc.sync.dma_start(
                        out=xt[:, dc, :],
                        in_=x[t0 : t0 + SEG_T, dc * 128 : (dc + 1) * 128],
                        transpose=True,
                    )

                # ---- Q.T / K.T projections (weights stationary, fp32r) ----
                qt = spool1.tile([128, 4, SEG_T], bf16, tag="qt")
                kt = spool1.tile([128, 4, SEG_T], bf16, tag="kt")
                for hc in range(8):
                    for half in range(2):
                        ps = pspool.tile([128, 400], f32, tag="ps")
                        for dc in range(4):
                            nc.tensor.matmul(
                                ps[:],
                                lhsT=wqk_sb[:, dc, hc * 128 : (hc + 1) * 128],
                                rhs=xt[:, dc, half * 400 : (half + 1) * 400],
                                start=(dc == 0),
                                stop=(dc == 3),
                            )
                        dst = qt if hc < 4 else kt
                        nc.vector.tensor_copy(
                            dst[:, hc % 4, half * 400 : (half + 1) * 400], ps[:]
                        )

                # ---- V natural (x.T stationary, fp32r) ----
                vsb = spool1.tile([TC, 8, DIM], bf16, tag="v")
                for k in range(8):
                    ps = pspool.tile([TC, DIM], f32, tag="ps")
                    for dc in range(4):
                        nc.tensor.matmul(
                            ps[:],
                            lhsT=xt[:, dc, k * TC : (k + 1) * TC],
                            rhs=wvt_sb[:, dc, :],
                            start=(dc == 0),
                            stop=(dc == 3),
                        )
                    nc.vector.tensor_copy(vsb[:, k, :], ps[:])

                # ---- attention per (block-in-seg, head) ----
                ot = spool1.tile([128, 4, SEG_T], bf16, tag="ot")
                for up in range(4):
                    ug = s * 4 + up
                    last = ug == NB - 1
                    jw = TC if last else C  # valid key count
                    njh = 1 if last else 2

                    # G = q @ Grev.T for all (head, ihalf) of this block
                    g_sb = bpool.tile([TC, 16, GW], bf16, tag="g")
                    for h in range(HEADS):
                        hp, hr = h // 2, (h % 2) * 64
                        for ih in range(2):
                            icol = up * C + ih * TC
                            psg = pspool.tile([TC, GW], f32, tag="ps")
                            nc.tensor.matmul(
                                psg[:],
                                lhsT=qt[hr : hr + 64, hp, icol : icol + TC],
                                rhs=grev_sb[hr : hr + 64, ih, :],
                                start=True,
                                stop=True,
                            )
                            nc.vector.tensor_copy(
                                g_sb[:, h * 2 + ih, :], psg[:]
                            )
                    gd = gsc[ug]
                    nc.sync.dma_start(
                        out=gd.rearrange("(c i s) -> i c s", c=16, s=GW),
                        in_=g_sb[:],
                    )
                    # sheared read-back: pos[i', c, j] = G[i', c, 99 - i' + j]
                    pos_sb = bpool.tile([TC, 16, C], bf16, tag="pos")
                    import concourse.bass as bass_mod

                    shear = bass_mod.AP(
                        gd.tensor,
                        gd.offset + 99,
                        [[GW - 1, TC], [TC * GW, 16], [1, C]],
                    )
                    nc.sync.dma_start(out=pos_sb[:], in_=shear)

                    for h in range(HEADS):
                        hp, hr = h // 2, (h % 2) * 64
                        p_tiles = []
                        den = wpool.tile([TC, 2], f32, tag="den")
                        rec = wpool.tile([TC, 2], f32, tag="rec")
                        for ih in range(2):
                            icol = up * C + ih * TC
                            qslice = qt[hr : hr + 64, hp, icol : icol + TC]
                            # dots (bf16)
                            psd = pspool.tile([TC, C], f32, tag="ps")
                            nc.tensor.matmul(
                                psd[:],
                                lhsT=qslice,
                                rhs=kt[hr : hr + 64, hp, up * C : (up + 1) * C],
                                start=True,
                                stop=True,
                            )
                            # logits = dots*scale + pos (pos pre-scaled on host)
                            lsb = wpool.tile([TC, C], f32, tag="L")
                            nc.vector.scalar_tensor_tensor(
                                out=lsb[:, 0:jw],
                                in0=psd[:, 0:jw],
                                scalar=SCALE,
                                in1=pos_sb[:, h * 2 + ih, 0:jw],
                                op0=mult,
                                op1=add,
                            )
                            # P = exp(logits), den = rowsum
                            psb = wpool.tile([TC, C], bf16, tag="P")
                            nc.scalar.activation(
                                out=psb[:, 0:jw],
                                in_=lsb[:, 0:jw],
                                func=Exp,
                                accum_out=den[:, ih : ih + 1],
                            )
                            p_tiles.append(psb)
                        nc.vector.reciprocal(rec[:], den[:])
                        pn_tiles = []
                        for ih in range(2):
                            pn = wpool.tile([TC, C], bf16, tag="pn")
                            nc.vector.tensor_scalar_mul(
                                pn[:, 0:jw],
                                p_tiles[ih][:, 0:jw],
                                rec[:, ih : ih + 1],
                            )
                            pn_tiles.append(pn)
                        p_tiles = pn_tiles

                        # P.T (PE transpose), batched PSUM evacuation
                        pspt = pspool.tile([TC, 2, C], bf16, tag="ps")
                        for jh in range(njh):
                            for ih in range(2):
                                nc.tensor.transpose(
                                    pspt[:, jh, ih * TC : (ih + 1) * TC],
                                    in_=p_tiles[ih][:, jh * TC : (jh + 1) * TC],
                                    identity=idb_sb[0:TC, 0:TC],
                                )
                        ptb = wpool.tile([TC, 2, C], bf16, tag="pt")
                        nc.vector.tensor_copy(
                            ptb[:, 0:njh, :], pspt[:, 0:njh, :]
                        )

                        # O.T = V.T @ P.T  (accumulate over key halves)
                        pso = pspool.tile([64, C], f32, tag="ps")
                        for jh in range(njh):
                            nc.tensor.matmul(
                                pso[:],
                                lhsT=vsb[:, up * 2 + jh, h * 64 : (h + 1) * 64],
                                rhs=ptb[:, jh, :],
                                start=(jh == 0),
                                stop=(jh == njh - 1),
                            )
                        nc.vector.tensor_copy(
                            ot[hr : hr + 64, hp, up * C : (up + 1) * C], pso[:]
                        )

                # ---- output projection (O.T stationary, fp32r) ----
                yt = spool2.tile([TC, 8, DIM], f32, tag="yt")
                nk = 8 if s < SEGS - 1 else 7
                for k in range(nk):
                    psy = pspool.tile([TC, DIM], f32, tag="ps")
                    for hdc in range(4):
                        nc.tensor.matmul(
                            psy[:],
                            lhsT=ot[:, hdc, k * TC : (k + 1) * TC],
                            rhs=wout_sb[:, hdc, :],
                            start=(hdc == 0),
                            stop=(hdc == 3),
                        )
                    nc.vector.tensor_add(yt[:, k, :], psy[:], bout_sb[0:TC, :])
                nc.sync.dma_start(
                    out=y[t0 : t0 + nk * TC, :].rearrange(
                        "(k p) d -> p k d", p=TC
                    ),
                    in_=yt[:, 0:nk, :],
                )

    nc.compile()
    return nc


def prep_inputs(x, Wq, Wkv, Wout, bout, rel_emb):
    """Host-side weight re-layouts + padding. Returns per-core in_maps."""
    import ml_dtypes

    x = np.asarray(x, dtype=np.float32)
    Wq = np.asarray(Wq, dtype=np.float32)
    Wkv = np.asarray(Wkv, dtype=np.float32)
    Wout = np.asarray(Wout, dtype=np.float32)
    bout = np.asarray(bout, dtype=np.float32)
    rel_emb = np.asarray(rel_emb, dtype=np.float32)

    bs = x.shape[0]
    bf = ml_dtypes.bfloat16
    xpad = np.zeros((bs, NP, DIM), dtype=bf)
    xpad[:, :N, :] = x.astype(bf)

    Wk = Wkv[:DIM]
    Wv = Wkv[DIM:]
    wqkt = np.ascontiguousarray(
        np.concatenate([Wq.T, Wk.T], axis=1)
    ).astype(bf)  # (512, 1024)
    wvt = np.ascontiguousarray(Wv.T).astype(bf)  # (512, 512)
    woutt = np.ascontiguousarray(Wout.T).astype(bf)  # (512, 512)

    # Grev[s] = rel_emb[711 - s] * scale; two per-ihalf windows of width GW
    grev = rel_emb[711 : 711 - 399 : -1] * SCALE  # (399, 64)
    grevt = np.zeros((128, 2, GW), dtype=bf)
    grevt[:DH, 0, :299] = grev[100:399].T
    grevt[:DH, 1, :304] = grev[0:304].T
    grevt[DH:, :, :] = grevt[:DH, :, :]

    idb = np.eye(128, dtype=np.float32).astype(ml_dtypes.bfloat16)
    boutb = np.ascontiguousarray(np.broadcast_to(bout, (128, DIM))).astype(
        np.float32
    )

    in_maps = []
    for b in range(bs):
        in_maps.append(
            dict(
                x=np.ascontiguousarray(xpad[b]),
                wqkt=wqkt,
                wvt=wvt,
                woutt=woutt,
                grevt=grevt,
                idb=idb,
                boutb=boutb,
            )
        )
    return in_maps


def kernel(x, Wq, Wkv, Wout, bout, rel_emb, context_size=200, **_):
    from concourse.bass_utils import run_bass_kernel_spmd

    in_maps = prep_inputs(x, Wq, Wkv, Wout, bout, rel_emb)
    nc = build_nc()
    res = run_bass_kernel_spmd(nc, in_maps, core_ids=list(range(8)))
    out = np.stack([res.results[b]["y"] for b in range(8)], axis=0)
    return out.astype(np.float32)


if __name__ == "__main__":
    nc = build_nc()
    print("built ok")


# revision 19
# speedup vs baseline: 3.7108x; 1.9528x over previous
"""Block-local sparse attention with relative position bias on 8 TRN2 NeuronCores.

Sharding: data-parallel over batch (bs=8 == n_cores). Core i computes batch i
end-to-end; weights are replicated. Inside each core the 20 attention blocks
(context_size=200, padded seq 4000) stream through SBUF in 5 segments of 800
tokens.

Hardcoded problem shapes (self-contained; no reference.py / spec.json reads):
  x (8, 3900, 512) f32, HEADS=8, DH=64, c=200, OFFSET=512.
"""

import math
import sys

import numpy as np

sys.path.insert(0, "/opt/trn_rl_repo")

HEADS = 8
DH = 64
DIM = 512
C = 200
N = 3900
NP = 4000
NB = 20
SEGS = 5
SEG_T = 800  # tokens per segment (4 blocks)
TC = 100  # token chunk (half block)
GW = 304  # padded G window width (299 -> 304)
SCALE = DH ** -0.5  # 0.125


def build_nc():
    import concourse.bass as bass
    import concourse.mybir as mybir
    import concourse.tile as tile
    from concourse import bacc

    f32 = mybir.dt.float32
    f32r = mybir.dt.float32r
    bf16 = mybir.dt.bfloat16
    Exp = mybir.ActivationFunctionType.Exp
    mult = mybir.AluOpType.mult
    add = mybir.AluOpType.add

    nc = bacc.Bacc("TRN2", target_bir_lowering=False, debug=False)

    x = nc.declare_dram_parameter("x", [NP, DIM], bf16, isOutput=False)
    wqkt = nc.declare_dram_parameter("wqkt", [DIM, 1024], bf16, isOutput=False)
    wvt = nc.declare_dram_parameter("wvt", [DIM, DIM], bf16, isOutput=False)
    woutt = nc.declare_dram_parameter("woutt", [DIM, DIM], bf16, isOutput=False)
    grevt = nc.declare_dram_parameter("grevt", [128, 2, GW], bf16, isOutput=False)
    idb = nc.declare_dram_parameter("idb", [128, 128], bf16, isOutput=False)
    boutb = nc.declare_dram_parameter("boutb", [128, DIM], f32, isOutput=False)
    y = nc.declare_dram_parameter("y", [N, DIM], f32, isOutput=True)

    # DRAM scratch for the relative-position shear (one slot per block/head/ihalf)
    gsc = nc.dram_tensor("gscratch", [NB, 16 * TC * GW], bf16)

    with tile.TileContext(nc) as tc:
        with (
            tc.tile_pool(name="const", bufs=1) as cpool,
            tc.tile_pool(name="seg1", bufs=2) as spool1,
            tc.tile_pool(name="seg2", bufs=2) as spool2,
            tc.tile_pool(name="work", bufs=4) as wpool,
            tc.tile_pool(name="hold", bufs=18) as hpool,
            tc.tile_pool(name="blk", bufs=2) as bpool,
            tc.tile_pool(name="psum", bufs=8, space="PSUM") as pspool,
        ):
            # ---- constants ----
            wqk_sb = cpool.tile([128, 4, 1024], bf16, tag="wqk")
            wvt_sb = cpool.tile([128, 4, DIM], bf16, tag="wvt")
            wout_sb = cpool.tile([128, 4, DIM], bf16, tag="wout")
            grev_sb = cpool.tile([128, 2, GW], bf16, tag="grev")
            idb_sb = cpool.tile([128, 128], bf16, tag="idb")
            bout_sb = cpool.tile([128, DIM], f32, tag="bout")

            for dc in range(4):
                r = slice(dc * 128, (dc + 1) * 128)
                nc.sync.dma_start(out=wqk_sb[:, dc, :], in_=wqkt[r, :])
                nc.sync.dma_start(out=wvt_sb[:, dc, :], in_=wvt[r, :])
                nc.sync.dma_start(out=wout_sb[:, dc, :], in_=woutt[r, :])
            nc.sync.dma_start(out=grev_sb[:], in_=grevt[:])
            nc.sync.dma_start(out=idb_sb[:], in_=idb[:])
            nc.sync.dma_start(out=bout_sb[:], in_=boutb[:])

            import concourse.bass as bass_mod

            pos_tiles = {}

            def emit_g(ug, qt):
                """G = q @ Grev.T for all (h, ih) of block ug; DMA out, sheared DMA back."""
                up = ug % 4
                g_sb = bpool.tile([TC, 16, GW], bf16, tag="g")
                for h in range(HEADS):
                    hp, hr = h // 2, (h % 2) * 64
                    for ih in range(2):
                        icol = up * C + ih * TC
                        psg = pspool.tile([TC, GW], f32, tag="ps")
                        nc.tensor.matmul(
                            psg[:],
                            lhsT=qt[hr : hr + 64, hp, icol : icol + TC],
                            rhs=grev_sb[hr : hr + 64, ih, :],
                            start=True,
                            stop=True,
                        )
                        if (h * 2 + ih) % 2 == 0:
                            nc.vector.tensor_copy(
                                g_sb[:, h * 2 + ih, :], psg[:]
                            )
                        else:
                            nc.scalar.copy(g_sb[:, h * 2 + ih, :], psg[:])
                gd = gsc[ug]
                nc.sync.dma_start(
                    out=gd.rearrange("(c i s) -> i c s", c=16, s=GW),
                    in_=g_sb[:],
                )
                pos_sb = bpool.tile([TC, 16, C], bf16, tag="pos")
                shear = bass_mod.AP(
                    gd.tensor,
                    gd.offset + 99,
                    [[GW - 1, TC], [TC * GW, 16], [1, C]],
                )
                nc.sync.dma_start(out=pos_sb[:], in_=shear)
                pos_tiles[ug] = pos_sb

            for s in range(SEGS):
                t0 = s * SEG_T

                # ---- x.T via hardware DMA-transpose from DRAM ----
                xt = spool1.tile([128, 4, SEG_T], bf16, tag="xt")
                for dc in range(4):
                    nc.sync.dma_start(
                        out=xt[:, dc, :],
                        in_=x[t0 : t0 + SEG_T, dc * 128 : (dc + 1) * 128],
                        transpose=True,
                    )

                # ---- Q.T / K.T projections (weights stationary) ----
                qt = spool1.tile([128, 4, SEG_T], bf16, tag="qt")
                kt = spool1.tile([128, 4, SEG_T], bf16, tag="kt")
                for hc in range(8):
                    for half in range(2):
                        ps = pspool.tile([128, 400], f32, tag="ps")
                        for dc in range(4):
                            nc.tensor.matmul(
                                ps[:],
                                lhsT=wqk_sb[:, dc, hc * 128 : (hc + 1) * 128],
                                rhs=xt[:, dc, half * 400 : (half + 1) * 400],
                                start=(dc == 0),
                                stop=(dc == 3),
                            )
                        dst = qt if hc < 4 else kt
                        nc.vector.tensor_copy(
                            dst[:, hc % 4, half * 400 : (half + 1) * 400], ps[:]
                        )

                # ---- V natural (x.T stationary) ----
                vsb = spool1.tile([TC, 8, DIM], bf16, tag="v")
                for k in range(8):
                    ps = pspool.tile([TC, DIM], f32, tag="ps")
                    for dc in range(4):
                        nc.tensor.matmul(
                            ps[:],
                            lhsT=xt[:, dc, k * TC : (k + 1) * TC],
                            rhs=wvt_sb[:, dc, :],
                            start=(dc == 0),
                            stop=(dc == 3),
                        )
                    nc.vector.tensor_copy(vsb[:, k, :], ps[:])

                # G for the first block of this segment (cross-seg blocks
                # can't prefetch earlier: they need this segment's qt)
                emit_g(s * 4, qt)

                ot = spool1.tile([128, 4, SEG_T], bf16, tag="ot")
                yt = spool2.tile([TC, 8, DIM], f32, tag="yt")
                for up in range(4):
                    ug = s * 4 + up
                    last = ug == NB - 1
                    jw = TC if last else C
                    njh = 1 if last else 2

                    # prefetch next block's G/pos during this block's attention
                    if up < 3:
                        emit_g(ug + 1, qt)

                    pos_sb = pos_tiles.pop(ug)

                    # phase A: dots + exp for all heads
                    p_all = []
                    for h in range(HEADS):
                        hp, hr = h // 2, (h % 2) * 64
                        den = hpool.tile([TC, 2], f32, tag="den")
                        p_pair = []
                        for ih in range(2):
                            icol = up * C + ih * TC
                            psd = pspool.tile([TC, C], f32, tag="ps")
                            nc.tensor.matmul(
                                psd[:],
                                lhsT=qt[hr : hr + 64, hp, icol : icol + TC],
                                rhs=kt[hr : hr + 64, hp, up * C : (up + 1) * C],
                                start=True,
                                stop=True,
                            )
                            lsb = wpool.tile([TC, C], f32, tag="L")
                            nc.vector.scalar_tensor_tensor(
                                out=lsb[:, 0:jw],
                                in0=psd[:, 0:jw],
                                scalar=SCALE,
                                in1=pos_sb[:, h * 2 + ih, 0:jw],
                                op0=mult,
                                op1=add,
                            )
                            psb = hpool.tile([TC, C], bf16, tag="P")
                            nc.scalar.activation(
                                out=psb[:, 0:jw],
                                in_=lsb[:, 0:jw],
                                func=Exp,
                                accum_out=den[:, ih : ih + 1],
                            )
                            p_pair.append(psb)
                        p_all.append((den, p_pair))

                    # phase B: normalize
                    pn_all = []
                    for h in range(HEADS):
                        den, p_pair = p_all[h]
                        rec = wpool.tile([TC, 2], f32, tag="rec")
                        nc.vector.reciprocal(rec[:], den[:])
                        pn_pair = []
                        for ih in range(2):
                            pn = hpool.tile([TC, C], bf16, tag="pn")
                            nc.vector.tensor_scalar_mul(
                                pn[:, 0:jw],
                                p_pair[ih][:, 0:jw],
                                rec[:, ih : ih + 1],
                            )
                            pn_pair.append(pn)
                        pn_all.append(pn_pair)

                    # phase C: P.T transposes
                    pt_all = []
                    for h in range(HEADS):
                        pspt = pspool.tile([TC, 2, C], bf16, tag="ps")
                        for jh in range(njh):
                            for ih in range(2):
                                nc.tensor.transpose(
                                    pspt[:, jh, ih * TC : (ih + 1) * TC],
                                    in_=pn_all[h][ih][:, jh * TC : (jh + 1) * TC],
                                    identity=idb_sb[0:TC, 0:TC],
                                )
                        ptb = hpool.tile([TC, 2, C], bf16, tag="pt")
                        nc.vector.tensor_copy(
                            ptb[:, 0:njh, :], pspt[:, 0:njh, :]
                        )
                        pt_all.append(ptb)

                    # phase D: O.T = V.T @ P.T
                    for h in range(HEADS):
                        hp, hr = h // 2, (h % 2) * 64
                        pso = pspool.tile([64, C], f32, tag="ps")
                        for jh in range(njh):
                            nc.tensor.matmul(
                                pso[:],
                                lhsT=vsb[:, up * 2 + jh, h * 64 : (h + 1) * 64],
                                rhs=pt_all[h][:, jh, :],
                                start=(jh == 0),
                                stop=(jh == njh - 1),
                            )
                        nc.vector.tensor_copy(
                            ot[hr : hr + 64, hp, up * C : (up + 1) * C], pso[:]
                        )

                    # phase E: output projection for this block's two chunks
                    for k in (2 * up, 2 * up + 1):
                        if t0 + k * TC >= N:
                            continue
                        psy = pspool.tile([TC, DIM], f32, tag="ps")
                        for hdc in range(4):
                            nc.tensor.matmul(
                                psy[:],
                                lhsT=ot[:, hdc, k * TC : (k + 1) * TC],
                                rhs=wout_sb[:, hdc, :],
                                start=(hdc == 0),
                                stop=(hdc == 3),
                            )
                        nc.vector.tensor_add(
                            yt[:, k, :], psy[:], bout_sb[0:TC, :]
                        )

                nk = 8 if s < SEGS - 1 else 7
                nc.sync.dma_start(
                    out=y[t0 : t0 + nk * TC, :].rearrange(
                        "(k p) d -> p k d", p=TC
                    ),
                    in_=yt[:, 0:nk, :],
                )

    nc.compile()
    return nc


def prep_inputs(x, Wq, Wkv, Wout, bout, rel_emb):
    """Host-side weight re-layouts + padding. Returns per-core in_maps."""
    import ml_dtypes

    x = np.asarray(x, dtype=np.float32)
    Wq = np.asarray(Wq, dtype=np.float32)
    Wkv = np.asarray(Wkv, dtype=np.float32)
    Wout = np.asarray(Wout, dtype=np.float32)
    bout = np.asarray(bout, dtype=np.float32)
    rel_emb = np.asarray(rel_emb, dtype=np.float32)

    bs = x.shape[0]
    bf = ml_dtypes.bfloat16
    xpad = np.zeros((bs, NP, DIM), dtype=bf)
    xpad[:, :N, :] = x.astype(bf)

    Wk = Wkv[:DIM]
    Wv = Wkv[DIM:]
    wqkt = np.ascontiguousarray(
        np.concatenate([Wq.T, Wk.T], axis=1)
    ).astype(bf)  # (512, 1024)
    wvt = np.ascontiguousarray(Wv.T).astype(bf)  # (512, 512)
    woutt = np.ascontiguousarray(Wout.T).astype(bf)  # (512, 512)

    # Grev[s] = rel_emb[711 - s] * scale; two per-ihalf windows of width GW
    grev = rel_emb[711 : 711 - 399 : -1] * SCALE  # (399, 64)
    grevt = np.zeros((128, 2, GW), dtype=bf)
    grevt[:DH, 0, :299] = grev[100:399].T
    grevt[:DH, 1, :304] = grev[0:304].T
    grevt[DH:, :, :] = grevt[:DH, :, :]

    idb = np.eye(128, dtype=np.float32).astype(ml_dtypes.bfloat16)
    boutb = np.ascontiguousarray(np.broadcast_to(bout, (128, DIM))).astype(
        np.float32
    )

    in_maps = []
    for b in range(bs):
        in_maps.append(
            dict(
                x=np.ascontiguousarray(xpad[b]),
                wqkt=wqkt,
                wvt=wvt,
                woutt=woutt,
                grevt=grevt,
                idb=idb,
                boutb=boutb,
            )
        )
    return in_maps


def kernel(x, Wq, Wkv, Wout, bout, rel_emb, context_size=200, **_):
    from concourse.bass_utils import run_bass_kernel_spmd

    in_maps = prep_inputs(x, Wq, Wkv, Wout, bout, rel_emb)
    nc = build_nc()
    res = run_bass_kernel_spmd(nc, in_maps, core_ids=list(range(8)))
    out = np.stack([res.results[b]["y"] for b in range(8)], axis=0)
    return out.astype(np.float32)


if __name__ == "__main__":
    nc = build_nc()
    print("built ok")


# revision 20
# speedup vs baseline: 3.7300x; 1.0052x over previous
"""Block-local sparse attention with relative position bias on 8 TRN2 NeuronCores.

Sharding: data-parallel over batch (bs=8 == n_cores). Core i computes batch i
end-to-end; weights are replicated. Inside each core the 20 attention blocks
(context_size=200, padded seq 4000) stream through SBUF in 5 segments of 800
tokens.

Hardcoded problem shapes (self-contained; no reference.py / spec.json reads):
  x (8, 3900, 512) f32, HEADS=8, DH=64, c=200, OFFSET=512.
"""

import math
import sys

import numpy as np

sys.path.insert(0, "/opt/trn_rl_repo")

HEADS = 8
DH = 64
DIM = 512
C = 200
N = 3900
NP = 4000
NB = 20
SEGS = 5
SEG_T = 800  # tokens per segment (4 blocks)
TC = 100  # token chunk (half block)
GW = 304  # padded G window width (299 -> 304)
SCALE = DH ** -0.5  # 0.125


def build_nc():
    import concourse.bass as bass
    import concourse.mybir as mybir
    import concourse.tile as tile
    from concourse import bacc

    f32 = mybir.dt.float32
    f32r = mybir.dt.float32r
    bf16 = mybir.dt.bfloat16
    Exp = mybir.ActivationFunctionType.Exp
    mult = mybir.AluOpType.mult
    add = mybir.AluOpType.add

    nc = bacc.Bacc("TRN2", target_bir_lowering=False, debug=False)

    x = nc.declare_dram_parameter("x", [NP, DIM], bf16, isOutput=False)
    wqkt = nc.declare_dram_parameter("wqkt", [DIM, 1024], bf16, isOutput=False)
    wvt = nc.declare_dram_parameter("wvt", [DIM, DIM], bf16, isOutput=False)
    woutt = nc.declare_dram_parameter("woutt", [DIM, DIM], bf16, isOutput=False)
    grevt = nc.declare_dram_parameter("grevt", [128, 2, GW], bf16, isOutput=False)
    idb = nc.declare_dram_parameter("idb", [128, 128], bf16, isOutput=False)
    boutb = nc.declare_dram_parameter("boutb", [128, DIM], f32, isOutput=False)
    y = nc.declare_dram_parameter("y", [N, DIM], f32, isOutput=True)

    # DRAM scratch for the relative-position shear (one slot per block/head/ihalf)
    gsc = nc.dram_tensor("gscratch", [NB, 16 * TC * GW], bf16)

    with tile.TileContext(nc) as tc:
        with (
            tc.tile_pool(name="const", bufs=1) as cpool,
            tc.tile_pool(name="seg1", bufs=2) as spool1,
            tc.tile_pool(name="seg2", bufs=2) as spool2,
            tc.tile_pool(name="work", bufs=4) as wpool,
            tc.tile_pool(name="hold", bufs=18) as hpool,
            tc.tile_pool(name="blk", bufs=2) as bpool,
            tc.tile_pool(name="psum", bufs=8, space="PSUM") as pspool,
        ):
            # ---- constants ----
            wqk_sb = cpool.tile([128, 4, 1024], bf16, tag="wqk")
            wvt_sb = cpool.tile([128, 4, DIM], bf16, tag="wvt")
            wout_sb = cpool.tile([128, 4, DIM], bf16, tag="wout")
            grev_sb = cpool.tile([128, 2, GW], bf16, tag="grev")
            idb_sb = cpool.tile([128, 128], bf16, tag="idb")
            bout_sb = cpool.tile([128, DIM], f32, tag="bout")

            for dc in range(4):
                r = slice(dc * 128, (dc + 1) * 128)
                nc.sync.dma_start(out=wqk_sb[:, dc, :], in_=wqkt[r, :])
                nc.sync.dma_start(out=wvt_sb[:, dc, :], in_=wvt[r, :])
                nc.sync.dma_start(out=wout_sb[:, dc, :], in_=woutt[r, :])
            nc.sync.dma_start(out=grev_sb[:], in_=grevt[:])
            nc.sync.dma_start(out=idb_sb[:], in_=idb[:])
            nc.sync.dma_start(out=bout_sb[:], in_=boutb[:])

            import concourse.bass as bass_mod

            pos_tiles = {}

            def emit_g(ug, qt):
                """G = q @ Grev.T for all (h, ih) of block ug; DMA out, sheared DMA back."""
                up = ug % 4
                g_sb = bpool.tile([TC, 16, GW], bf16, tag="g")
                for h in range(HEADS):
                    hp, hr = h // 2, (h % 2) * 64
                    for ih in range(2):
                        icol = up * C + ih * TC
                        psg = pspool.tile([TC, GW], f32, tag="ps")
                        nc.tensor.matmul(
                            psg[:],
                            lhsT=qt[hr : hr + 64, hp, icol : icol + TC],
                            rhs=grev_sb[hr : hr + 64, ih, :],
                            start=True,
                            stop=True,
                        )
                        nc.scalar.copy(g_sb[:, h * 2 + ih, :], psg[:])
                gd = gsc[ug]
                nc.sync.dma_start(
                    out=gd.rearrange("(c i s) -> i c s", c=16, s=GW),
                    in_=g_sb[:],
                )
                pos_sb = bpool.tile([TC, 16, C], bf16, tag="pos")
                shear = bass_mod.AP(
                    gd.tensor,
                    gd.offset + 99,
                    [[GW - 1, TC], [TC * GW, 16], [1, C]],
                )
                nc.sync.dma_start(out=pos_sb[:], in_=shear)
                pos_tiles[ug] = pos_sb

            for s in range(SEGS):
                t0 = s * SEG_T

                # ---- x.T via hardware DMA-transpose from DRAM ----
                xt = spool1.tile([128, 4, SEG_T], bf16, tag="xt")
                for dc in range(4):
                    nc.sync.dma_start(
                        out=xt[:, dc, :],
                        in_=x[t0 : t0 + SEG_T, dc * 128 : (dc + 1) * 128],
                        transpose=True,
                    )

                # ---- Q.T / K.T projections (weights stationary) ----
                qt = spool1.tile([128, 4, SEG_T], bf16, tag="qt")
                kt = spool1.tile([128, 4, SEG_T], bf16, tag="kt")
                for hc in range(8):
                    for half in range(2):
                        ps = pspool.tile([128, 400], f32, tag="ps")
                        for dc in range(4):
                            nc.tensor.matmul(
                                ps[:],
                                lhsT=wqk_sb[:, dc, hc * 128 : (hc + 1) * 128],
                                rhs=xt[:, dc, half * 400 : (half + 1) * 400],
                                start=(dc == 0),
                                stop=(dc == 3),
                            )
                        dst = qt if hc < 4 else kt
                        nc.vector.tensor_copy(
                            dst[:, hc % 4, half * 400 : (half + 1) * 400], ps[:]
                        )

                # ---- V natural (x.T stationary) ----
                vsb = spool1.tile([TC, 8, DIM], bf16, tag="v")
                for k in range(8):
                    ps = pspool.tile([TC, DIM], f32, tag="ps")
                    for dc in range(4):
                        nc.tensor.matmul(
                            ps[:],
                            lhsT=xt[:, dc, k * TC : (k + 1) * TC],
                            rhs=wvt_sb[:, dc, :],
                            start=(dc == 0),
                            stop=(dc == 3),
                        )
                    nc.vector.tensor_copy(vsb[:, k, :], ps[:])

                # G for the first block of this segment (cross-seg blocks
                # can't prefetch earlier: they need this segment's qt)
                emit_g(s * 4, qt)

                ot = spool1.tile([128, 4, SEG_T], bf16, tag="ot")
                yt = spool2.tile([TC, 8, DIM], f32, tag="yt")
                for up in range(4):
                    ug = s * 4 + up
                    last = ug == NB - 1
                    jw = TC if last else C
                    njh = 1 if last else 2

                    # prefetch next block's G/pos during this block's attention
                    if up < 3:
                        emit_g(ug + 1, qt)

                    pos_sb = pos_tiles.pop(ug)

                    # phase A: dots + exp for all heads
                    p_all = []
                    for h in range(HEADS):
                        hp, hr = h // 2, (h % 2) * 64
                        den = hpool.tile([TC, 2], f32, tag="den")
                        p_pair = []
                        for ih in range(2):
                            icol = up * C + ih * TC
                            psd = pspool.tile([TC, C], f32, tag="ps")
                            nc.tensor.matmul(
                                psd[:],
                                lhsT=qt[hr : hr + 64, hp, icol : icol + TC],
                                rhs=kt[hr : hr + 64, hp, up * C : (up + 1) * C],
                                start=True,
                                stop=True,
                            )
                            lsb = wpool.tile([TC, C], f32, tag="L")
                            nc.vector.scalar_tensor_tensor(
                                out=lsb[:, 0:jw],
                                in0=psd[:, 0:jw],
                                scalar=SCALE,
                                in1=pos_sb[:, h * 2 + ih, 0:jw],
                                op0=mult,
                                op1=add,
                            )
                            psb = hpool.tile([TC, C], bf16, tag="P")
                            nc.scalar.activation(
                                out=psb[:, 0:jw],
                                in_=lsb[:, 0:jw],
                                func=Exp,
                                accum_out=den[:, ih : ih + 1],
                            )
                            p_pair.append(psb)
                        p_all.append((den, p_pair))

                    # phase B: normalize
                    pn_all = []
                    for h in range(HEADS):
                        den, p_pair = p_all[h]
                        rec = wpool.tile([TC, 2], f32, tag="rec")
                        nc.vector.reciprocal(rec[:], den[:])
                        pn_pair = []
                        for ih in range(2):
                            pn = hpool.tile([TC, C], bf16, tag="pn")
                            nc.vector.tensor_scalar_mul(
                                pn[:, 0:jw],
                                p_pair[ih][:, 0:jw],
                                rec[:, ih : ih + 1],
                            )
                            pn_pair.append(pn)
                        pn_all.append(pn_pair)

                    # phase C: P.T transposes
                    pt_all = []
                    for h in range(HEADS):
                        pspt = pspool.tile([TC, 2, C], bf16, tag="ps")
                        for jh in range(njh):
                            for ih in range(2):
                                nc.tensor.transpose(
                                    pspt[:, jh, ih * TC : (ih + 1) * TC],
                                    in_=pn_all[h][ih][:, jh * TC : (jh + 1) * TC],
                                    identity=idb_sb[0:TC, 0:TC],
                                )
                        ptb = hpool.tile([TC, 2, C], bf16, tag="pt")
                        nc.vector.tensor_copy(
                            ptb[:, 0:njh, :], pspt[:, 0:njh, :]
                        )
                        pt_all.append(ptb)

                    # phase D: O.T = V.T @ P.T
                    for h in range(HEADS):
                        hp, hr = h // 2, (h % 2) * 64
                        pso = pspool.tile([64, C], f32, tag="ps")
                        for jh in range(njh):
                            nc.tensor.matmul(
                                pso[:],
                                lhsT=vsb[:, up * 2 + jh, h * 64 : (h + 1) * 64],
                                rhs=pt_all[h][:, jh, :],
                                start=(jh == 0),
                                stop=(jh == njh - 1),
                            )
                        nc.vector.tensor_copy(
                            ot[hr : hr + 64, hp, up * C : (up + 1) * C], pso[:]
                        )

                    # phase E: output projection for this block's two chunks
                    for k in (2 * up, 2 * up + 1):
                        if t0 + k * TC >= N:
                            continue
                        psy = pspool.tile([TC, DIM], f32, tag="ps")
                        for hdc in range(4):
                            nc.tensor.matmul(
                                psy[:],
                                lhsT=ot[:, hdc, k * TC : (k + 1) * TC],
                                rhs=wout_sb[:, hdc, :],
                                start=(hdc == 0),
                                stop=(hdc == 3),
                            )
                        nc.vector.tensor_add(
                            yt[:, k, :], psy[:], bout_sb[0:TC, :]
                        )

                nk = 8 if s < SEGS - 1 else 7
                nc.sync.dma_start(
                    out=y[t0 : t0 + nk * TC, :].rearrange(
                        "(k p) d -> p k d", p=TC
                    ),
                    in_=yt[:, 0:nk, :],
                )

    nc.compile()
    return nc


def prep_inputs(x, Wq, Wkv, Wout, bout, rel_emb):
    """Host-side weight re-layouts + padding. Returns per-core in_maps."""
    import ml_dtypes

    x = np.asarray(x, dtype=np.float32)
    Wq = np.asarray(Wq, dtype=np.float32)
    Wkv = np.asarray(Wkv, dtype=np.float32)
    Wout = np.asarray(Wout, dtype=np.float32)
    bout = np.asarray(bout, dtype=np.float32)
    rel_emb = np.asarray(rel_emb, dtype=np.float32)

    bs = x.shape[0]
    bf = ml_dtypes.bfloat16
    xpad = np.zeros((bs, NP, DIM), dtype=bf)
    xpad[:, :N, :] = x.astype(bf)

    Wk = Wkv[:DIM]
    Wv = Wkv[DIM:]
    wqkt = np.ascontiguousarray(
        np.concatenate([Wq.T, Wk.T], axis=1)
    ).astype(bf)  # (512, 1024)
    wvt = np.ascontiguousarray(Wv.T).astype(bf)  # (512, 512)
    woutt = np.ascontiguousarray(Wout.T).astype(bf)  # (512, 512)

    # Grev[s] = rel_emb[711 - s] * scale; two per-ihalf windows of width GW
    grev = rel_emb[711 : 711 - 399 : -1] * SCALE  # (399, 64)
    grevt = np.zeros((128, 2, GW), dtype=bf)
    grevt[:DH, 0, :299] = grev[100:399].T
    grevt[:DH, 1, :304] = grev[0:304].T
    grevt[DH:, :, :] = grevt[:DH, :, :]

    idb = np.eye(128, dtype=np.float32).astype(ml_dtypes.bfloat16)
    boutb = np.ascontiguousarray(np.broadcast_to(bout, (128, DIM))).astype(
        np.float32
    )

    in_maps = []
    for b in range(bs):
        in_maps.append(
            dict(
                x=np.ascontiguousarray(xpad[b]),
                wqkt=wqkt,
                wvt=wvt,
                woutt=woutt,
                grevt=grevt,
                idb=idb,
                boutb=boutb,
            )
        )
    return in_maps


def kernel(x, Wq, Wkv, Wout, bout, rel_emb, context_size=200, **_):
    from concourse.bass_utils import run_bass_kernel_spmd

    in_maps = prep_inputs(x, Wq, Wkv, Wout, bout, rel_emb)
    nc = build_nc()
    res = run_bass_kernel_spmd(nc, in_maps, core_ids=list(range(8)))
    out = np.stack([res.results[b]["y"] for b in range(8)], axis=0)
    return out.astype(np.float32)


if __name__ == "__main__":
    nc = build_nc()
    print("built ok")
